# revision 32
# baseline (speedup 1.0000x reference)
"""Trainium2 Bass kernel for nn_BatchATSSAssigner (ATSS label assignment).

Strategy (8 NeuronCores, pure data parallelism over batch B=32 -> 4 images/core):

Per core, partitions are used two ways:
  * "PBG" layout: partition = prior-within-chunk (128), free = (66 chunks, 128 bg)
    where bg = img*32 + gt.  All elementwise IoU / threshold / mask work.
  * "BGP" layout: partition = bg (128), free = prior p = chunk*128 + r (8448).
    Only for the per-level top-9 selection (DVE max8 + match_replace).

Highlights:
  * top-9-smallest-distance per level == top-9-largest of -d^2.
  * candidate threshold (mean + std ddof=1 of the 27 candidate IoUs) via
    TensorE ones-matmul column sums of mask*ovl and (mask*ovl)^2.
  * multi-gt resolution: argmax one-hot == (ovl == rowmax).
  * labels/boxes/fg gathered by one block-diagonal matmul (pos is one-hot).
  * the (B,G,P) pred-IoU tensor of the reference collapses to IoU(pred_p,
    assigned_gt_p) computed only on the gathered boxes.
  * outputs written in device-native [r, chunk, img, ...] layout with
    partition-contiguous DMA descriptors; host reshapes to (B, P, ...).

All numerics validated bit-exact against the jax reference (see mimic.py);
on hardware the only deviation is reciprocal-vs-divide ulp noise (~4e-8).
"""

from contextlib import ExitStack

import numpy as np

import concourse.bass as bass
import concourse.bacc as bacc
import concourse.tile as tile
from concourse import mybir
from concourse.bass_utils import run_bass_kernel_spmd

F32 = mybir.dt.float32
BF16 = mybir.dt.bfloat16
I32 = mybir.dt.int32
U8 = mybir.dt.uint8
ALU = mybir.AluOpType
AF = mybir.ActivationFunctionType
AX = mybir.AxisListType

B, G, P = 32, 32, 8400
NCORES = 8
BPC = B // NCORES          # images per core
BG = BPC * G               # 128
NCH = 66                   # chunks of 128 priors
PPAD = NCH * 128           # 8448
NCLS = 80
LEVELS = ((0, 6400), (6400, 1600), (8000, 400))
NEGBIG = -1.0e30
EPS = 1e-9
R27 = float(np.float32(1.0) / np.float32(27.0))
R26 = float(np.float32(1.0) / np.float32(26.0))

# packed constant-input layout: name -> (start, end) column in cst [128, CST_W]
_CST_FIELDS = [
    ("pr", NCH * 10),
    ("gtt", 8 * BG),
    ("w", 6 * BPC),
    ("pad", 1),
    ("box0", 4 * BPC),
    ("predt", NCH * BPC * 4),
    ("preda", NCH * BPC),
    ("iota80", NCLS),
    ("ident", 128),
]
CST_OFF = {}
_off = 0
for _n, _wd in _CST_FIELDS:
    CST_OFF[_n] = (_off, _off + _wd)
    _off += _wd
CST_W = _off


def _build_module():
    nc = bacc.Bacc("TRN2", target_bir_lowering=False, debug=False)

    # All small inputs are packed into ONE tensor so a single dma_start (one
    # DMAHW sem lane) covers them: most ISA structs can only encode one
    # sync-wait, so consumers must not need waits on two DMA lanes.
    cst = nc.dram_tensor("cst", [128, CST_W], F32, kind="ExternalInput")

    labels_o = nc.dram_tensor("labels_o", [128, NCH, BPC], I32, kind="ExternalOutput")
    boxes_o = nc.dram_tensor("boxes_o", [128, NCH, BPC, 4], F32, kind="ExternalOutput")
    scores_o = nc.dram_tensor(
        "scores_o", [128, NCH, BPC, NCLS], F32, kind="ExternalOutput"
    )
    fg_o = nc.dram_tensor("fg_o", [128, NCH, BPC], U8, kind="ExternalOutput")

    with tile.TileContext(nc) as tc:
        with ExitStack() as ctx:
            _kernel(ctx, tc, cst, labels_o, boxes_o, scores_o, fg_o)
    nc.compile()
    return nc


def _kernel(ctx, tc, cst, labels_o, boxes_o, scores_o, fg_o):
    nc = tc.nc

    consts = ctx.enter_context(tc.tile_pool(name="consts", bufs=1))
    bigp = ctx.enter_context(tc.tile_pool(name="bigp", bufs=1))
    smallp = ctx.enter_context(tc.tile_pool(name="smallp", bufs=1))
    stage = ctx.enter_context(tc.tile_pool(name="stage", bufs=2))
    scst = ctx.enter_context(tc.tile_pool(name="scst", bufs=6))
    pst = ctx.enter_context(tc.tile_pool(name="pst", bufs=2, space="PSUM"))
    psr = ctx.enter_context(tc.tile_pool(name="psr", bufs=1, space="PSUM"))
    psg = ctx.enter_context(tc.tile_pool(name="psg", bufs=3, space="PSUM"))

    # ---- load all constants / prepped inputs with ONE DMA ----
    cst_sb = consts.tile([128, CST_W], F32, tag="cst")
    nc.sync.dma_start(cst_sb[:], cst[:])

    def cslice(name, shape):
        a, b = CST_OFF[name]
        v = cst_sb[:, a:b]
        if len(shape) == 2:
            return v
        if len(shape) == 3:
            return v.rearrange("p (a b) -> p a b", b=shape[2])
        return v.rearrange("p (a b c) -> p a b c", b=shape[2], c=shape[3])

    pr_sb = cslice("pr", [128, NCH, 10])
    gtt_sb = cslice("gtt", [128, 8, BG])
    w_sb = cslice("w", [128, 6 * BPC])
    pad_sb = cslice("pad", [128, 1])
    box0_sb = cslice("box0", [128, 4 * BPC])
    predt_sb = cslice("predt", [128, NCH, BPC, 4])
    preda_sb = cslice("preda", [128, NCH, BPC])
    iota_sb = cslice("iota80", [128, NCLS])
    ident_sb = cslice("ident", [128, 128])

    ones_col = consts.tile([128, 1], F32, tag="ones_col")
    nc.vector.memset(ones_col[:], 1.0)
    ones_row = consts.tile([1, 128], F32, tag="ones_row")
    nc.vector.memset(ones_row[:], 1.0)

    # gt broadcast tables (each [128, BG], identical rows)
    gx1b = gtt_sb[:, 0, :]
    gy1b = gtt_sb[:, 1, :]
    gx2b = gtt_sb[:, 2, :]
    gy2b = gtt_sb[:, 3, :]
    areagb = gtt_sb[:, 4, :]
    gcxb = gtt_sb[:, 5, :]
    gcyb = gtt_sb[:, 6, :]

    def prs(c, j):  # prior scalar column [128,1] for chunk c
        return pr_sb[:, c, j:j + 1]

    # big working tiles; slots are reused across phases (Tile serializes)
    bt1 = bigp.tile([128, NCH, BG], F32, tag="bt1")
    bt2 = bigp.tile([128, NCH, BG], F32, tag="bt2")
    bt3 = bigp.tile([128, PPAD], F32, tag="bt3")
    candT = bigp.tile([128, NCH, BG], BF16, tag="bt4")
    ovl = bigp.tile([128, NCH, BG], F32, tag="bt5")
    ingts = bigp.tile([128, NCH, BG], U8, tag="bt6")
    bt1f = bt1[:].rearrange("p c b -> p (c b)")
    bt2f = bt2[:].rearrange("p c b -> p (c b)")
    bt3v = bt3[:].rearrange("p (c b) -> p c b", b=BG)

    # ====== phase 0: center-strictly-inside-gt mask (input-only work) ======
    # x deltas+min into bt1 (DVE); y1 delta via ACT Identity-bias into bt5;
    # y2 delta folded with a per-chunk stt (no buffer).  DVE never blocks:
    # the d^2 chunk loop below fills the window while ACT produces y1.
    for c in range(NCH):
        nc.vector.tensor_scalar(bt1[:, c, :], gx1b, prs(c, 4), -1.0,
                                ALU.subtract, ALU.mult)
        nc.vector.scalar_tensor_tensor(bt1[:, c, :], gx2b, prs(c, 4),
                                       bt1[:, c, :], ALU.subtract, ALU.min)
        nc.scalar.activation(ovl[:, c, :], gy1b, AF.Identity,
                             bias=prs(c, 5), scale=-1.0)

    # ======= phase 1: d^2 (dx in bt2, dy/d2/negd2 in bt3, in place) ========
    for c in range(NCH):
        nc.vector.tensor_scalar(bt2[:, c, :], gcxb, prs(c, 4), None, ALU.subtract)
        nc.vector.tensor_scalar(bt3v[:, c, :], gcyb, prs(c, 5), None, ALU.subtract)
    # finish ingts while ACT squares run
    nc.vector.tensor_tensor(bt1[:], bt1[:], ovl[:], ALU.min)  # min with y1
    for c in range(NCH):
        nc.vector.scalar_tensor_tensor(bt1[:, c, :], gy2b, prs(c, 9),
                                       bt1[:, c, :], ALU.add, ALU.min)
    nc.vector.tensor_scalar(ingts[:], bt1[:], EPS, None, ALU.is_gt)

    nc.scalar.square(bt2[:], bt2[:])
    nc.scalar.square(bt3v[:], bt3v[:])
    nc.vector.tensor_tensor(bt3v[:], bt2[:], bt3v[:], ALU.add)  # d2

    # transpose to BGP with negation, in place: same columns hold chunk c in
    # both layouts.  negd2[bg, c*128+r] = -d2[r, c, bg]
    for c0 in range(0, NCH, 4):
        n4 = min(4, NCH - c0)
        ps = pst.tile([128, 512], F32, tag="ps_t")
        for j in range(n4):
            nc.tensor.transpose(ps[:, j * 128:(j + 1) * 128],
                                bt3v[:, c0 + j, :], ident_sb)
        nc.scalar.mul(bt3[:, c0 * 128:(c0 + n4) * 128], ps[:, :n4 * 128], -1.0)

    # ================= phase 2: top-9 per level (BGP) =================
    # work = bt1 flat view; negd2 in bt3.  Runs before the IoU phase so that
    # the threshold PE sums can stream inside the IoU phase afterwards.
    m8 = smallp.tile([128, 8], F32, tag="m8")
    for (start, n) in LEVELS:
        nl = bt3[:, start:start + n]
        wl = bt1f[:, start:start + n]
        # iteration 1: find+zap top-8 (writes the modified rows into work)
        nc.vector.max(m8[:], nl)
        nc.vector.match_replace(wl, m8[:], nl, NEGBIG)
        # iteration 2: zap the 9th
        nc.vector.max(m8[:], wl)
        nc.vector.memset(m8[:, 1:8], NEGBIG)
        nc.vector.match_replace(wl, m8[:], wl, NEGBIG)
    # mask = (negd2 != work) * pad; pad columns cleared
    nc.vector.tensor_tensor(bt2f, bt3[:], bt1f, ALU.not_equal)
    nc.vector.tensor_scalar(bt2f, bt2f, pad_sb, None, ALU.mult)
    nc.vector.memset(bt2f[:, P:PPAD], 0.0)

    # transpose candidate mask back to PBG (bf16: exact for 0/1)
    for c0 in range(0, NCH, 4):
        n4 = min(4, NCH - c0)
        ps = pst.tile([128, 512], F32, tag="ps_t")
        for j in range(n4):
            c = c0 + j
            nc.tensor.transpose(ps[:, j * 128:(j + 1) * 128],
                                bt2f[:, c * 128:(c + 1) * 128], ident_sb)
        nc.scalar.copy(candT[:, c0:c0 + n4, :], ps[:, :n4 * 128])

    # ====== phases 3+4: IoU overlaps with threshold sums streamed in =======
    # Per quarter: wx/wy -> inter -> union -> recip -> ovl -> maskovl -> sq,
    # then the S1/S2 accumulating matmuls for that quarter run on PE while
    # the DVE moves on to the next quarter.
    s1p = psr.tile([1, 512], F32, tag="ps_row")
    s2p = psr.tile([1, 512], F32, tag="ps_row2")
    QTRS = [(0, 16), (16, 32), (32, 48), (48, 66)]
    sub4 = [(c0, min(4, NCH - c0)) for c0 in range(0, NCH, 4)]
    first_sub = sub4[0][0]
    last_sub = sub4[-1][0]
    for (qa, qb) in QTRS:
        qs = slice(qa, qb)
        qf = slice(qa * 128, qb * 128)
        for c in range(qa, qb):
            nc.vector.tensor_scalar(bt1[:, c, :], gx1b, prs(c, 0), None, ALU.max)
            nc.vector.scalar_tensor_tensor(bt1[:, c, :], gx2b, prs(c, 2),
                                           bt1[:, c, :], ALU.min, ALU.subtract)
            nc.vector.tensor_scalar(bt2[:, c, :], gy1b, prs(c, 1), None, ALU.max)
            nc.vector.scalar_tensor_tensor(bt2[:, c, :], gy2b, prs(c, 3),
                                           bt2[:, c, :], ALU.min, ALU.subtract)
        nc.scalar.activation(bt2f[:, qf], bt2f[:, qf], AF.Relu)
        # inter = relu(wx) * relu(wy) -> bt2
        nc.vector.scalar_tensor_tensor(bt2f[:, qf], bt1f[:, qf], 0.0,
                                       bt2f[:, qf], ALU.max, ALU.mult)
        # union = area_g + area_p - inter -> bt1 (1e-6 clamp never binds)
        for c in range(qa, qb):
            nc.vector.scalar_tensor_tensor(bt1[:, c, :], areagb, prs(c, 6),
                                           bt2[:, c, :], ALU.add, ALU.subtract)
        nc.vector.reciprocal(bt1f[:, qf], bt1f[:, qf])
        nc.vector.tensor_tensor(ovl[:, qs, :], bt2[:, qs, :], bt1[:, qs, :],
                                ALU.mult)
        # maskovl -> bt1, its square -> bt2
        nc.vector.tensor_tensor(bt1[:, qs, :], ovl[:, qs, :], candT[:, qs, :],
                                ALU.mult)
        nc.scalar.square(bt2f[:, qf], bt1f[:, qf])
        for c0, n4 in [s for s in sub4 if qa <= s[0] < qb]:
            nc.tensor.matmul(s1p[:, :n4 * 128], ones_col[:],
                             bt1f[:, c0 * 128:(c0 + n4) * 128],
                             start=(c0 == first_sub), stop=(c0 == last_sub),
                             skip_group_check=True)
            nc.tensor.matmul(s2p[:, :n4 * 128], ones_col[:],
                             bt2f[:, c0 * 128:(c0 + n4) * 128],
                             start=(c0 == first_sub), stop=(c0 == last_sub),
                             skip_group_check=True)

    # multi-gt prep overlaps the tail of the PE sums; eq lives in bt3
    ovl4 = ovl[:].rearrange("p c (i g) -> p c i g", g=G)
    rmax = smallp.tile([128, NCH, BPC], F32, tag="rmax")
    nc.vector.tensor_reduce(rmax[:], ovl4, AX.X, ALU.max)
    eq4 = bt3v[:, :, :].rearrange("p c (i g) -> p c i g", g=G)
    rmax_bc = rmax[:, :, :, None].to_broadcast([128, NCH, BPC, G])
    nc.vector.tensor_tensor(eq4, ovl4, rmax_bc, ALU.is_equal)

    # threshold row math (tiny) + broadcast; fold the 4 column-partials first
    s1w = smallp.tile([1, 512], F32, tag="s1w")
    nc.scalar.copy(s1w[:], s1p[:])
    s2w = smallp.tile([1, 512], F32, tag="s2w")
    nc.scalar.copy(s2w[:], s2p[:])
    s1row = smallp.tile([1, BG], F32, tag="s1row")
    s2row = smallp.tile([1, BG], F32, tag="s2row")
    nc.vector.tensor_tensor(s1row[:], s1w[:, 0:128], s1w[:, 128:256], ALU.add)
    nc.vector.tensor_tensor(s1row[:], s1row[:], s1w[:, 256:384], ALU.add)
    nc.vector.tensor_tensor(s1row[:], s1row[:], s1w[:, 384:512], ALU.add)
    nc.vector.tensor_tensor(s2row[:], s2w[:, 0:128], s2w[:, 128:256], ALU.add)
    nc.vector.tensor_tensor(s2row[:], s2row[:], s2w[:, 256:384], ALU.add)
    nc.vector.tensor_tensor(s2row[:], s2row[:], s2w[:, 384:512], ALU.add)
    meanrow = smallp.tile([1, BG], F32, tag="meanrow")
    nc.vector.tensor_scalar(meanrow[:], s1row[:], R27, None, ALU.mult)
    varrow = smallp.tile([1, BG], F32, tag="varrow")
    nc.vector.tensor_tensor(varrow[:], s1row[:], meanrow[:], ALU.mult)
    nc.vector.tensor_tensor(varrow[:], s2row[:], varrow[:], ALU.subtract)
    nc.vector.tensor_scalar(varrow[:], varrow[:], R26, 0.0, ALU.mult, ALU.max)
    nc.scalar.sqrt(varrow[:], varrow[:])
    thrrow = smallp.tile([1, BG], F32, tag="thrrow")
    nc.vector.tensor_tensor(thrrow[:], meanrow[:], varrow[:], ALU.add)
    thr_ps = psr.tile([128, BG], F32, tag="ps_bc")
    nc.tensor.matmul(thr_ps[:], ones_row[:], thrrow[:], start=True, stop=True)
    thrb = smallp.tile([128, BG], F32, tag="thrb")
    nc.scalar.copy(thrb[:], thr_ps[:])

    # ================= phase 6: pos_pre =================
    # pos = (maskovl > thr) & in_gts; the explicit cand factor is redundant:
    # maskovl = ovl*cand, and maskovl > thr iff cand & (ovl > thr) (thr >= 0,
    # and at thr == 0 both reduce to cand & (ovl > 0)).
    pos = bt1  # maskovl slot, updated in place
    thr_bc = thrb[:, None, :].to_broadcast([128, NCH, BG])
    nc.vector.tensor_tensor(pos[:], pos[:], thr_bc, ALU.is_gt)
    nc.vector.tensor_tensor(pos[:], pos[:], ingts[:], ALU.mult)

    # ================= phase 7: multi-gt resolution =================
    pos4 = pos[:].rearrange("p c (i g) -> p c i g", g=G)
    fgp = smallp.tile([128, NCH, BPC], F32, tag="fgp")
    nc.vector.tensor_reduce(fgp[:], pos4, AX.X, ALU.add)
    multi = smallp.tile([128, NCH, BPC], U8, tag="multi")
    nc.vector.tensor_scalar(multi[:], fgp[:], 1.0, None, ALU.is_gt)
    multi_bc = multi[:, :, :, None].to_broadcast([128, NCH, BPC, G])
    nc.vector.copy_predicated(pos4, multi_bc, eq4)

    # ========== phases 8-10: gather -> post -> scores, pipelined ==========
    # Processed in 4 chunk-groups so scores/DMAs for early chunks overlap
    # the gather matmuls of later chunks.
    gath = smallp.tile([128, NCH, BPC, 6], F32, tag="gath")
    fg1m = smallp.tile([128, NCH, BPC], F32, tag="fg1m")
    labf = smallp.tile([128, NCH, BPC], F32, tag="labf")
    lab_i32 = smallp.tile([128, NCH, BPC], I32, tag="lab_i32")
    fg_u8 = smallp.tile([128, NCH, BPC], U8, tag="fg_u8")
    boxs = smallp.tile([128, NCH, BPC, 4], F32, tag="boxs")
    # iou scratch carved out of bt2 (dead after the S2 matmul reads)
    iw = bt2[:, :, 0:BPC]
    ih = bt2[:, :, BPC:2 * BPC]
    tq = bt2[:, :, 2 * BPC:3 * BPC]
    i2 = bt2[:, :, 3 * BPC:4 * BPC]
    ag = bt2[:, :, 4 * BPC:5 * BPC]
    iouu = bt2[:, :, 5 * BPC:6 * BPC]

    GROUPS = [(0, 16), (16, 32), (32, 48), (48, 66)]
    for (ca, cb) in GROUPS:
        # gather this group's chunks: gath[:, c, i, f]
        for c0 in range(ca, cb, 4):
            n4 = min(4, cb - c0)
            ps = pst.tile([128, 512], F32, tag="ps_t")
            posT = stage.tile([128, 512], F32, tag="posT")
            gps = psg.tile([128, 4, BPC, 6], F32, tag="ps_g")
            for j in range(n4):
                nc.tensor.transpose(ps[:, j * 128:(j + 1) * 128],
                                    pos[:, c0 + j, :], ident_sb)
            nc.scalar.copy(posT[:, :n4 * 128], ps[:, :n4 * 128])
            for j in range(n4):
                nc.tensor.matmul(gps[:, j, :, :],
                                 posT[:, j * 128:(j + 1) * 128], w_sb,
                                 start=True, stop=True)
            nc.scalar.copy(gath[:, c0:c0 + n4, :, :], gps[:, :n4, :, :])

        sl = slice(ca, cb)
        fg = gath[:, sl, :, 5]
        lab = gath[:, sl, :, 4]

        def gf(f):
            return gath[:, sl, :, f]

        def qf(f):
            return predt_sb[:, sl, :, f]

        nc.vector.tensor_scalar(fg1m[:, sl], fg, -1.0, 1.0, ALU.mult, ALU.add)
        # labels_final = gathered_label + (1-fg)*80
        nc.vector.scalar_tensor_tensor(labf[:, sl], fg1m[:, sl], 80.0, lab,
                                       ALU.mult, ALU.add)
        nc.vector.tensor_copy(lab_i32[:, sl], labf[:, sl])
        nc.scalar.dma_start(labels_o[:, sl], lab_i32[:, sl])
        nc.gpsimd.tensor_copy(fg_u8[:, sl], fg)
        nc.scalar.dma_start(fg_o[:, sl], fg_u8[:, sl])
        # boxes = gathered_box + (1-fg)*box0
        for i in range(BPC):
            for f in range(4):
                nc.vector.scalar_tensor_tensor(
                    boxs[:, sl, i, f], fg1m[:, sl, i],
                    box0_sb[:, 4 * i + f:4 * i + f + 1],
                    gath[:, sl, i, f], ALU.mult, ALU.add)
        nc.sync.dma_start(boxes_o[:, sl], boxs[:, sl])

        # IoU(pred, assigned gt)
        nc.vector.tensor_tensor(iw[:, sl], gf(0), qf(0), ALU.max)
        nc.vector.tensor_tensor(tq[:, sl], gf(2), qf(2), ALU.min)
        nc.vector.tensor_tensor(iw[:, sl], tq[:, sl], iw[:, sl], ALU.subtract)
        nc.vector.tensor_tensor(ih[:, sl], gf(1), qf(1), ALU.max)
        nc.vector.tensor_tensor(tq[:, sl], gf(3), qf(3), ALU.min)
        nc.vector.tensor_tensor(ih[:, sl], tq[:, sl], ih[:, sl], ALU.subtract)
        nc.vector.tensor_scalar(ih[:, sl], ih[:, sl], 0.0, None, ALU.max)
        nc.vector.scalar_tensor_tensor(i2[:, sl], iw[:, sl], 0.0, ih[:, sl],
                                       ALU.max, ALU.mult)
        # gathered gt area (clipped)
        nc.vector.tensor_tensor(ag[:, sl], gf(2), gf(0), ALU.subtract)
        nc.vector.tensor_tensor(tq[:, sl], gf(3), gf(1), ALU.subtract)
        nc.vector.tensor_scalar(tq[:, sl], tq[:, sl], 0.0, None, ALU.max)
        nc.vector.scalar_tensor_tensor(ag[:, sl], ag[:, sl], 0.0, tq[:, sl],
                                       ALU.max, ALU.mult)
        # denom = ag + ap - i2 + EPS ; iou = i2 / denom * fg
        nc.vector.tensor_tensor(ag[:, sl], ag[:, sl], preda_sb[:, sl], ALU.add)
        nc.vector.tensor_tensor(ag[:, sl], ag[:, sl], i2[:, sl], ALU.subtract)
        nc.vector.tensor_scalar(ag[:, sl], ag[:, sl], EPS, None, ALU.add)
        nc.vector.reciprocal(ag[:, sl], ag[:, sl])
        nc.vector.tensor_tensor(i2[:, sl], i2[:, sl], ag[:, sl], ALU.mult)
        nc.vector.tensor_tensor(iouu[:, sl], i2[:, sl], fg, ALU.mult)

        # scores
        for c in range(ca, cb):
            sc = scst.tile([128, BPC, NCLS], F32, tag="sc")
            for i in range(BPC):
                eng = nc.vector if i < 2 else nc.gpsimd
                eng.tensor_scalar(
                    sc[:, i, :], iota_sb, labf[:, c, i:i + 1],
                    iouu[:, c, i:i + 1], ALU.is_equal, ALU.mult)
            deng = nc.sync if (c % 2 == 0) else nc.scalar
            deng.dma_start(scores_o[:, c, :, :], sc[:])


# ---------------- host-side glue ----------------

_CACHE: dict = {}


def _prep_core_inputs(priors, gt_bboxes, gt_labels, pad_flag, pred_bboxes):
    """Build per-core input dicts (numpy only reshapes/replicates inputs)."""
    # priors -> [128, NCH, 10]: x1,y1,x2,y2,cx,cy,area,0,-cx,-cy (pad 1e6)
    ppad = np.full((PPAD, 4), 1.0e6, np.float32)
    ppad[:P] = priors
    px1, py1, px2, py2 = ppad[:, 0], ppad[:, 1], ppad[:, 2], ppad[:, 3]
    pcx = (px1 + px2) / np.float32(2.0)
    pcy = (py1 + py2) / np.float32(2.0)
    areap = (px2 - px1) * (py2 - py1)
    pr = np.stack([px1, py1, px2, py2, pcx, pcy, areap,
                   np.zeros_like(px1), -pcx, -pcy], axis=1)   # (PPAD, 10)
    pr = pr.reshape(NCH, 128, 10).transpose(1, 0, 2).copy()   # [r, c, 10]

    iota = np.broadcast_to(np.arange(NCLS, dtype=np.float32), (128, NCLS)).copy()
    ident = np.eye(128, dtype=np.float32)

    ins = []
    for core in range(NCORES):
        i0 = core * BPC
        gtb = gt_bboxes[i0:i0 + BPC].reshape(BG, 4).astype(np.float32)
        lab = gt_labels[i0:i0 + BPC].reshape(BG).astype(np.float32)
        padf = pad_flag[i0:i0 + BPC].reshape(BG, 1).astype(np.float32)
        gx1, gy1, gx2, gy2 = gtb[:, 0], gtb[:, 1], gtb[:, 2], gtb[:, 3]
        gcx = (gx1 + gx2) / np.float32(2.0)
        gcy = (gy1 + gy2) / np.float32(2.0)
        areag = (gx2 - gx1) * (gy2 - gy1)
        rows = np.stack([gx1, gy1, gx2, gy2, areag, gcx, gcy,
                         np.zeros_like(gx1)], axis=0)     # (8, BG)
        gtt = np.broadcast_to(rows[None], (128, 8, BG)).copy()

        # block-diag gather weights [BG, 6*BPC]: per img: x1,y1,x2,y2,label,1
        w = np.zeros((BG, 6 * BPC), np.float32)
        for i in range(BPC):
            sl = slice(i * G, (i + 1) * G)
            w[sl, 6 * i + 0:6 * i + 4] = gtb[sl]
            w[sl, 6 * i + 4] = lab[sl]
            w[sl, 6 * i + 5] = 1.0

        b0 = gt_bboxes[i0:i0 + BPC, 0, :].astype(np.float32).reshape(-1)  # (16,)
        box0 = np.broadcast_to(b0[None], (128, 4 * BPC)).copy()

        pb = pred_bboxes[i0:i0 + BPC].astype(np.float32)  # (BPC, P, 4)
        pbp = np.zeros((BPC, PPAD, 4), np.float32)
        pbp[:, :P] = pb
        predt = pbp.reshape(BPC, NCH, 128, 4).transpose(2, 1, 0, 3).copy()
        pa = (np.clip(pbp[..., 2] - pbp[..., 0], 0, None)
              * np.clip(pbp[..., 3] - pbp[..., 1], 0, None))  # (BPC, PPAD)
        preda = pa.reshape(BPC, NCH, 128).transpose(2, 1, 0).copy()

        parts = {
            "pr": pr, "gtt": gtt, "w": w, "pad": padf, "box0": box0,
            "predt": predt, "preda": preda, "iota80": iota, "ident": ident,
        }
        cstk = np.zeros((128, CST_W), np.float32)
        for name, _w in _CST_FIELDS:
            a, b = CST_OFF[name]
            cstk[:, a:b] = parts[name].reshape(128, b - a)
        ins.append({"cst": cstk})
    return ins


def kernel(priors, num_level_priors, gt_labels, gt_bboxes, pad_bbox_flag,
           pred_bboxes, _want_trace=False):
    priors = np.asarray(priors, np.float32)
    gt_bboxes = np.asarray(gt_bboxes, np.float32)
    gt_labels = np.asarray(gt_labels).reshape(B, G)
    pad_flag = np.asarray(pad_bbox_flag, np.float32).reshape(B, G)
    pred_bboxes = np.asarray(pred_bboxes, np.float32)

    if "nc" not in _CACHE:
        _CACHE["nc"] = _build_module()
    nc = _CACHE["nc"]

    in_maps = _prep_core_inputs(priors, gt_bboxes, gt_labels, pad_flag,
                                pred_bboxes)
    res = run_bass_kernel_spmd(nc, in_maps, core_ids=list(range(NCORES)),
                               trace=_want_trace)

    labels = np.empty((B, P), np.int32)
    boxes = np.empty((B, P, 4), np.float32)
    scores = np.empty((B, P, NCLS), np.float32)
    fg = np.empty((B, P), bool)
    for core in range(NCORES):
        r = res.results[core]
        i0 = core * BPC
        labv = r["labels_o"].transpose(2, 1, 0).reshape(BPC, PPAD)
        labels[i0:i0 + BPC] = labv[:, :P]
        bx = r["boxes_o"].transpose(2, 1, 0, 3).reshape(BPC, PPAD, 4)
        boxes[i0:i0 + BPC] = bx[:, :P]
        sc = r["scores_o"].transpose(2, 1, 0, 3).reshape(BPC, PPAD, NCLS)
        scores[i0:i0 + BPC] = sc[:, :P]
        f = r["fg_o"].transpose(2, 1, 0).reshape(BPC, PPAD)
        fg[i0:i0 + BPC] = f[:, :P] > 0
    out = (labels, boxes, scores, fg)
    if _want_trace:
        return out, res
    return out


# revision 33
# speedup vs baseline: 1.0473x; 1.0473x over previous
"""Trainium2 Bass kernel for nn_BatchATSSAssigner (ATSS label assignment).

Strategy (8 NeuronCores, pure data parallelism over batch B=32 -> 4 images/core):

Per core, partitions are used two ways:
  * "PBG" layout: partition = prior-within-chunk (128), free = (66 chunks, 128 bg)
    where bg = img*32 + gt.  All elementwise IoU / threshold / mask work.
  * "BGP" layout: partition = bg (128), free = prior p = chunk*128 + r (8448).
    Only for the per-level top-9 selection (DVE max8 + match_replace).

Highlights:
  * top-9-smallest-distance per level == top-9-largest of -d^2.
  * candidate threshold (mean + std ddof=1 of the 27 candidate IoUs) via
    TensorE ones-matmul column sums of mask*ovl and (mask*ovl)^2.
  * multi-gt resolution: argmax one-hot == (ovl == rowmax).
  * labels/boxes/fg gathered by one block-diagonal matmul (pos is one-hot).
  * the (B,G,P) pred-IoU tensor of the reference collapses to IoU(pred_p,
    assigned_gt_p) computed only on the gathered boxes.
  * outputs written in device-native [r, chunk, img, ...] layout with
    partition-contiguous DMA descriptors; host reshapes to (B, P, ...).

All numerics validated bit-exact against the jax reference (see mimic.py);
on hardware the only deviation is reciprocal-vs-divide ulp noise (~4e-8).
"""

from contextlib import ExitStack

import numpy as np

import concourse.bass as bass
import concourse.bacc as bacc
import concourse.tile as tile
from concourse import mybir
from concourse.bass_utils import run_bass_kernel_spmd

F32 = mybir.dt.float32
BF16 = mybir.dt.bfloat16
I32 = mybir.dt.int32
U8 = mybir.dt.uint8
ALU = mybir.AluOpType
AF = mybir.ActivationFunctionType
AX = mybir.AxisListType

B, G, P = 32, 32, 8400
NCORES = 8
BPC = B // NCORES          # images per core
BG = BPC * G               # 128
NCH = 66                   # chunks of 128 priors
PPAD = NCH * 128           # 8448
NCLS = 80
LEVELS = ((0, 6400), (6400, 1600), (8000, 400))
NEGBIG = -1.0e30
EPS = 1e-9
R27 = float(np.float32(1.0) / np.float32(27.0))
R26 = float(np.float32(1.0) / np.float32(26.0))

# packed constant-input layout: name -> (start, end) column in cst [128, CST_W]
_CST_FIELDS = [
    ("pr", NCH * 10),
    ("gtt", 8 * BG),
    ("w", 6 * BPC),
    ("pad", 1),
    ("box0", 4 * BPC),
    ("predt", NCH * BPC * 4),
    ("preda", NCH * BPC),
    ("iota80", NCLS),
    ("ident", 128),
]
CST_OFF = {}
_off = 0
for _n, _wd in _CST_FIELDS:
    CST_OFF[_n] = (_off, _off + _wd)
    _off += _wd
CST_W = _off


def _build_module():
    nc = bacc.Bacc("TRN2", target_bir_lowering=False, debug=False)

    # All small inputs are packed into ONE tensor so a single dma_start (one
    # DMAHW sem lane) covers them: most ISA structs can only encode one
    # sync-wait, so consumers must not need waits on two DMA lanes.
    cst = nc.dram_tensor("cst", [128, CST_W], F32, kind="ExternalInput")

    labels_o = nc.dram_tensor("labels_o", [128, NCH, BPC], I32, kind="ExternalOutput")
    boxes_o = nc.dram_tensor("boxes_o", [128, NCH, BPC, 4], F32, kind="ExternalOutput")
    scores_o = nc.dram_tensor(
        "scores_o", [128, NCH, BPC, NCLS], F32, kind="ExternalOutput"
    )
    fg_o = nc.dram_tensor("fg_o", [128, NCH, BPC], U8, kind="ExternalOutput")

    with tile.TileContext(nc) as tc:
        with ExitStack() as ctx:
            _kernel(ctx, tc, cst, labels_o, boxes_o, scores_o, fg_o)
    nc.compile()
    return nc


def _kernel(ctx, tc, cst, labels_o, boxes_o, scores_o, fg_o):
    nc = tc.nc

    consts = ctx.enter_context(tc.tile_pool(name="consts", bufs=1))
    bigp = ctx.enter_context(tc.tile_pool(name="bigp", bufs=1))
    smallp = ctx.enter_context(tc.tile_pool(name="smallp", bufs=1))
    stage = ctx.enter_context(tc.tile_pool(name="stage", bufs=2))
    scst = ctx.enter_context(tc.tile_pool(name="scst", bufs=6))
    pst = ctx.enter_context(tc.tile_pool(name="pst", bufs=2, space="PSUM"))
    psr = ctx.enter_context(tc.tile_pool(name="psr", bufs=1, space="PSUM"))
    psg = ctx.enter_context(tc.tile_pool(name="psg", bufs=3, space="PSUM"))

    # ---- load all constants / prepped inputs with ONE DMA ----
    cst_sb = consts.tile([128, CST_W], F32, tag="cst")
    nc.sync.dma_start(cst_sb[:], cst[:])

    def cslice(name, shape):
        a, b = CST_OFF[name]
        v = cst_sb[:, a:b]
        if len(shape) == 2:
            return v
        if len(shape) == 3:
            return v.rearrange("p (a b) -> p a b", b=shape[2])
        return v.rearrange("p (a b c) -> p a b c", b=shape[2], c=shape[3])

    pr_sb = cslice("pr", [128, NCH, 10])
    gtt_sb = cslice("gtt", [128, 8, BG])
    w_sb = cslice("w", [128, 6 * BPC])
    pad_sb = cslice("pad", [128, 1])
    box0_sb = cslice("box0", [128, 4 * BPC])
    predt_sb = cslice("predt", [128, NCH, BPC, 4])
    preda_sb = cslice("preda", [128, NCH, BPC])
    iota_sb = cslice("iota80", [128, NCLS])
    ident_sb = cslice("ident", [128, 128])

    ones_col = consts.tile([128, 1], F32, tag="ones_col")
    nc.vector.memset(ones_col[:], 1.0)
    ones_row = consts.tile([1, 128], F32, tag="ones_row")
    nc.vector.memset(ones_row[:], 1.0)

    # gt broadcast tables (each [128, BG], identical rows)
    gx1b = gtt_sb[:, 0, :]
    gy1b = gtt_sb[:, 1, :]
    gx2b = gtt_sb[:, 2, :]
    gy2b = gtt_sb[:, 3, :]
    areagb = gtt_sb[:, 4, :]
    gcxb = gtt_sb[:, 5, :]
    gcyb = gtt_sb[:, 6, :]

    def prs(c, j):  # prior scalar column [128,1] for chunk c
        return pr_sb[:, c, j:j + 1]

    # big working tiles; slots are reused across phases (Tile serializes)
    bt1 = bigp.tile([128, NCH, BG], F32, tag="bt1")
    bt2 = bigp.tile([128, NCH, BG], F32, tag="bt2")
    bt3 = bigp.tile([128, PPAD], F32, tag="bt3")
    candT = bigp.tile([128, NCH, BG], BF16, tag="bt4")
    ovl = bigp.tile([128, NCH, BG], F32, tag="bt5")
    ingts = bigp.tile([128, NCH, BG], U8, tag="bt6")
    bt1f = bt1[:].rearrange("p c b -> p (c b)")
    bt2f = bt2[:].rearrange("p c b -> p (c b)")
    bt3v = bt3[:].rearrange("p (c b) -> p c b", b=BG)

    # ====== phase 0: center-strictly-inside-gt mask (input-only work) ======
    # x deltas+min into bt1 (DVE); y1 delta via ACT Identity-bias into bt5;
    # y2 delta folded with a per-chunk stt (no buffer).  DVE never blocks:
    # the d^2 chunk loop below fills the window while ACT produces y1.
    for c in range(NCH):
        nc.vector.tensor_scalar(bt1[:, c, :], gx1b, prs(c, 4), -1.0,
                                ALU.subtract, ALU.mult)
        nc.vector.scalar_tensor_tensor(bt1[:, c, :], gx2b, prs(c, 4),
                                       bt1[:, c, :], ALU.subtract, ALU.min)
        nc.scalar.activation(ovl[:, c, :], gy1b, AF.Identity,
                             bias=prs(c, 5), scale=-1.0)

    # ======= phase 1: d^2 (dx in bt2, dy/d2/negd2 in bt3, in place) ========
    for c in range(NCH):
        nc.vector.tensor_scalar(bt2[:, c, :], gcxb, prs(c, 4), None, ALU.subtract)
        nc.gpsimd.tensor_scalar(bt3v[:, c, :], gcyb, prs(c, 5), None, ALU.subtract)
    # finish ingts while ACT squares run
    nc.vector.tensor_tensor(bt1[:], bt1[:], ovl[:], ALU.min)  # min with y1
    for c in range(NCH):
        nc.vector.scalar_tensor_tensor(bt1[:, c, :], gy2b, prs(c, 9),
                                       bt1[:, c, :], ALU.add, ALU.min)
    nc.vector.tensor_scalar(ingts[:], bt1[:], EPS, None, ALU.is_gt)

    nc.scalar.square(bt2[:], bt2[:])
    nc.scalar.square(bt3v[:], bt3v[:])
    nc.vector.tensor_tensor(bt3v[:], bt2[:], bt3v[:], ALU.add)  # d2

    # transpose to BGP with negation, in place: same columns hold chunk c in
    # both layouts.  negd2[bg, c*128+r] = -d2[r, c, bg]
    for c0 in range(0, NCH, 4):
        n4 = min(4, NCH - c0)
        ps = pst.tile([128, 512], F32, tag="ps_t")
        for j in range(n4):
            nc.tensor.transpose(ps[:, j * 128:(j + 1) * 128],
                                bt3v[:, c0 + j, :], ident_sb)
        nc.scalar.mul(bt3[:, c0 * 128:(c0 + n4) * 128], ps[:, :n4 * 128], -1.0)

    # ================= phase 2: top-9 per level (BGP) =================
    # work = bt1 flat view; negd2 in bt3.  Runs before the IoU phase so that
    # the threshold PE sums can stream inside the IoU phase afterwards.
    m8 = smallp.tile([128, 8], F32, tag="m8")
    for (start, n) in LEVELS:
        nl = bt3[:, start:start + n]
        wl = bt1f[:, start:start + n]
        # iteration 1: find+zap top-8 (writes the modified rows into work)
        nc.vector.max(m8[:], nl)
        nc.vector.match_replace(wl, m8[:], nl, NEGBIG)
        # iteration 2: zap the 9th
        nc.vector.max(m8[:], wl)
        nc.vector.memset(m8[:, 1:8], NEGBIG)
        nc.vector.match_replace(wl, m8[:], wl, NEGBIG)
    # mask = (negd2 != work) * pad; pad columns cleared
    nc.vector.tensor_tensor(bt2f, bt3[:], bt1f, ALU.not_equal)
    nc.vector.tensor_scalar(bt2f, bt2f, pad_sb, None, ALU.mult)
    nc.vector.memset(bt2f[:, P:PPAD], 0.0)

    # transpose candidate mask back to PBG (bf16: exact for 0/1)
    for c0 in range(0, NCH, 4):
        n4 = min(4, NCH - c0)
        ps = pst.tile([128, 512], F32, tag="ps_t")
        for j in range(n4):
            c = c0 + j
            nc.tensor.transpose(ps[:, j * 128:(j + 1) * 128],
                                bt2f[:, c * 128:(c + 1) * 128], ident_sb)
        nc.scalar.copy(candT[:, c0:c0 + n4, :], ps[:, :n4 * 128])

    # ====== phases 3+4: IoU overlaps with threshold sums streamed in =======
    # Per quarter: wx/wy -> inter -> union -> recip -> ovl -> maskovl -> sq,
    # then the S1/S2 accumulating matmuls for that quarter run on PE while
    # the DVE moves on to the next quarter.
    s1p = psr.tile([1, 512], F32, tag="ps_row")
    s2p = psr.tile([1, 512], F32, tag="ps_row2")
    QTRS = [(0, 16), (16, 32), (32, 48), (48, 66)]
    sub4 = [(c0, min(4, NCH - c0)) for c0 in range(0, NCH, 4)]
    first_sub = sub4[0][0]
    last_sub = sub4[-1][0]
    for (qa, qb) in QTRS:
        qs = slice(qa, qb)
        qf = slice(qa * 128, qb * 128)
        for c in range(qa, qb):
            nc.vector.tensor_scalar(bt1[:, c, :], gx1b, prs(c, 0), None, ALU.max)
            nc.vector.scalar_tensor_tensor(bt1[:, c, :], gx2b, prs(c, 2),
                                           bt1[:, c, :], ALU.min, ALU.subtract)
            nc.vector.tensor_scalar(bt2[:, c, :], gy1b, prs(c, 1), None, ALU.max)
            nc.vector.scalar_tensor_tensor(bt2[:, c, :], gy2b, prs(c, 3),
                                           bt2[:, c, :], ALU.min, ALU.subtract)
        nc.scalar.activation(bt2f[:, qf], bt2f[:, qf], AF.Relu)
        # inter = relu(wx) * relu(wy) -> bt2
        nc.vector.scalar_tensor_tensor(bt2f[:, qf], bt1f[:, qf], 0.0,
                                       bt2f[:, qf], ALU.max, ALU.mult)
        # union = area_g + area_p - inter -> bt1 (1e-6 clamp never binds)
        for c in range(qa, qb):
            nc.vector.scalar_tensor_tensor(bt1[:, c, :], areagb, prs(c, 6),
                                           bt2[:, c, :], ALU.add, ALU.subtract)
        nc.vector.reciprocal(bt1f[:, qf], bt1f[:, qf])
        nc.vector.tensor_tensor(ovl[:, qs, :], bt2[:, qs, :], bt1[:, qs, :],
                                ALU.mult)
        # maskovl -> bt1, its square -> bt2
        nc.vector.tensor_tensor(bt1[:, qs, :], ovl[:, qs, :], candT[:, qs, :],
                                ALU.mult)
        nc.scalar.square(bt2f[:, qf], bt1f[:, qf])
        for c0, n4 in [s for s in sub4 if qa <= s[0] < qb]:
            nc.tensor.matmul(s1p[:, :n4 * 128], ones_col[:],
                             bt1f[:, c0 * 128:(c0 + n4) * 128],
                             start=(c0 == first_sub), stop=(c0 == last_sub),
                             skip_group_check=True)
            nc.tensor.matmul(s2p[:, :n4 * 128], ones_col[:],
                             bt2f[:, c0 * 128:(c0 + n4) * 128],
                             start=(c0 == first_sub), stop=(c0 == last_sub),
                             skip_group_check=True)

    # multi-gt prep overlaps the tail of the PE sums; eq lives in bt3
    ovl4 = ovl[:].rearrange("p c (i g) -> p c i g", g=G)
    rmax = smallp.tile([128, NCH, BPC], F32, tag="rmax")
    nc.vector.tensor_reduce(rmax[:], ovl4, AX.X, ALU.max)
    eq4 = bt3v[:, :, :].rearrange("p c (i g) -> p c i g", g=G)
    rmax_bc = rmax[:, :, :, None].to_broadcast([128, NCH, BPC, G])
    nc.vector.tensor_tensor(eq4, ovl4, rmax_bc, ALU.is_equal)

    # threshold row math (tiny) + broadcast; fold the 4 column-partials first
    s1w = smallp.tile([1, 512], F32, tag="s1w")
    nc.scalar.copy(s1w[:], s1p[:])
    s2w = smallp.tile([1, 512], F32, tag="s2w")
    nc.scalar.copy(s2w[:], s2p[:])
    s1row = smallp.tile([1, BG], F32, tag="s1row")
    s2row = smallp.tile([1, BG], F32, tag="s2row")
    nc.vector.tensor_tensor(s1row[:], s1w[:, 0:128], s1w[:, 128:256], ALU.add)
    nc.vector.tensor_tensor(s1row[:], s1row[:], s1w[:, 256:384], ALU.add)
    nc.vector.tensor_tensor(s1row[:], s1row[:], s1w[:, 384:512], ALU.add)
    nc.vector.tensor_tensor(s2row[:], s2w[:, 0:128], s2w[:, 128:256], ALU.add)
    nc.vector.tensor_tensor(s2row[:], s2row[:], s2w[:, 256:384], ALU.add)
    nc.vector.tensor_tensor(s2row[:], s2row[:], s2w[:, 384:512], ALU.add)
    meanrow = smallp.tile([1, BG], F32, tag="meanrow")
    nc.vector.tensor_scalar(meanrow[:], s1row[:], R27, None, ALU.mult)
    varrow = smallp.tile([1, BG], F32, tag="varrow")
    nc.vector.tensor_tensor(varrow[:], s1row[:], meanrow[:], ALU.mult)
    nc.vector.tensor_tensor(varrow[:], s2row[:], varrow[:], ALU.subtract)
    nc.vector.tensor_scalar(varrow[:], varrow[:], R26, 0.0, ALU.mult, ALU.max)
    nc.scalar.sqrt(varrow[:], varrow[:])
    thrrow = smallp.tile([1, BG], F32, tag="thrrow")
    nc.vector.tensor_tensor(thrrow[:], meanrow[:], varrow[:], ALU.add)
    thr_ps = psr.tile([128, BG], F32, tag="ps_bc")
    nc.tensor.matmul(thr_ps[:], ones_row[:], thrrow[:], start=True, stop=True)
    thrb = smallp.tile([128, BG], F32, tag="thrb")
    nc.scalar.copy(thrb[:], thr_ps[:])

    # ================= phase 6: pos_pre =================
    # pos = (maskovl > thr) & in_gts; the explicit cand factor is redundant:
    # maskovl = ovl*cand, and maskovl > thr iff cand & (ovl > thr) (thr >= 0,
    # and at thr == 0 both reduce to cand & (ovl > 0)).
    pos = bt1  # maskovl slot, updated in place
    thr_bc = thrb[:, None, :].to_broadcast([128, NCH, BG])
    nc.vector.tensor_tensor(pos[:], pos[:], thr_bc, ALU.is_gt)
    nc.vector.tensor_tensor(pos[:], pos[:], ingts[:], ALU.mult)

    # ================= phase 7: multi-gt resolution =================
    pos4 = pos[:].rearrange("p c (i g) -> p c i g", g=G)
    fgp = smallp.tile([128, NCH, BPC], F32, tag="fgp")
    nc.vector.tensor_reduce(fgp[:], pos4, AX.X, ALU.add)
    multi = smallp.tile([128, NCH, BPC], U8, tag="multi")
    nc.vector.tensor_scalar(multi[:], fgp[:], 1.0, None, ALU.is_gt)
    multi_bc = multi[:, :, :, None].to_broadcast([128, NCH, BPC, G])
    nc.vector.copy_predicated(pos4, multi_bc, eq4)

    # ========== phases 8-10: gather -> post -> scores, pipelined ==========
    # Processed in 4 chunk-groups so scores/DMAs for early chunks overlap
    # the gather matmuls of later chunks.
    gath = smallp.tile([128, NCH, BPC, 6], F32, tag="gath")
    fg1m = smallp.tile([128, NCH, BPC], F32, tag="fg1m")
    labf = smallp.tile([128, NCH, BPC], F32, tag="labf")
    lab_i32 = smallp.tile([128, NCH, BPC], I32, tag="lab_i32")
    fg_u8 = smallp.tile([128, NCH, BPC], U8, tag="fg_u8")
    boxs = smallp.tile([128, NCH, BPC, 4], F32, tag="boxs")
    # iou scratch carved out of bt2 (dead after the S2 matmul reads)
    iw = bt2[:, :, 0:BPC]
    ih = bt2[:, :, BPC:2 * BPC]
    tq = bt2[:, :, 2 * BPC:3 * BPC]
    i2 = bt2[:, :, 3 * BPC:4 * BPC]
    ag = bt2[:, :, 4 * BPC:5 * BPC]
    iouu = bt2[:, :, 5 * BPC:6 * BPC]

    GROUPS = [(0, 16), (16, 32), (32, 48), (48, 66)]
    for (ca, cb) in GROUPS:
        # gather this group's chunks: gath[:, c, i, f]
        for c0 in range(ca, cb, 4):
            n4 = min(4, cb - c0)
            ps = pst.tile([128, 512], F32, tag="ps_t")
            posT = stage.tile([128, 512], F32, tag="posT")
            gps = psg.tile([128, 4, BPC, 6], F32, tag="ps_g")
            for j in range(n4):
                nc.tensor.transpose(ps[:, j * 128:(j + 1) * 128],
                                    pos[:, c0 + j, :], ident_sb)
            nc.scalar.copy(posT[:, :n4 * 128], ps[:, :n4 * 128])
            for j in range(n4):
                nc.tensor.matmul(gps[:, j, :, :],
                                 posT[:, j * 128:(j + 1) * 128], w_sb,
                                 start=True, stop=True)
            nc.scalar.copy(gath[:, c0:c0 + n4, :, :], gps[:, :n4, :, :])

        sl = slice(ca, cb)
        fg = gath[:, sl, :, 5]
        lab = gath[:, sl, :, 4]

        def gf(f):
            return gath[:, sl, :, f]

        def qf(f):
            return predt_sb[:, sl, :, f]

        nc.vector.tensor_scalar(fg1m[:, sl], fg, -1.0, 1.0, ALU.mult, ALU.add)
        # labels_final = gathered_label + (1-fg)*80
        nc.vector.scalar_tensor_tensor(labf[:, sl], fg1m[:, sl], 80.0, lab,
                                       ALU.mult, ALU.add)
        nc.vector.tensor_copy(lab_i32[:, sl], labf[:, sl])
        nc.scalar.dma_start(labels_o[:, sl], lab_i32[:, sl])
        nc.gpsimd.tensor_copy(fg_u8[:, sl], fg)
        nc.scalar.dma_start(fg_o[:, sl], fg_u8[:, sl])
        # boxes = gathered_box + (1-fg)*box0
        for i in range(BPC):
            for f in range(4):
                nc.vector.scalar_tensor_tensor(
                    boxs[:, sl, i, f], fg1m[:, sl, i],
                    box0_sb[:, 4 * i + f:4 * i + f + 1],
                    gath[:, sl, i, f], ALU.mult, ALU.add)
        nc.sync.dma_start(boxes_o[:, sl], boxs[:, sl])

        # IoU(pred, assigned gt)
        nc.vector.tensor_tensor(iw[:, sl], gf(0), qf(0), ALU.max)
        nc.vector.tensor_tensor(tq[:, sl], gf(2), qf(2), ALU.min)
        nc.vector.tensor_tensor(iw[:, sl], tq[:, sl], iw[:, sl], ALU.subtract)
        nc.vector.tensor_tensor(ih[:, sl], gf(1), qf(1), ALU.max)
        nc.vector.tensor_tensor(tq[:, sl], gf(3), qf(3), ALU.min)
        nc.vector.tensor_tensor(ih[:, sl], tq[:, sl], ih[:, sl], ALU.subtract)
        nc.vector.tensor_scalar(ih[:, sl], ih[:, sl], 0.0, None, ALU.max)
        nc.vector.scalar_tensor_tensor(i2[:, sl], iw[:, sl], 0.0, ih[:, sl],
                                       ALU.max, ALU.mult)
        # gathered gt area (clipped)
        nc.vector.tensor_tensor(ag[:, sl], gf(2), gf(0), ALU.subtract)
        nc.vector.tensor_tensor(tq[:, sl], gf(3), gf(1), ALU.subtract)
        nc.vector.tensor_scalar(tq[:, sl], tq[:, sl], 0.0, None, ALU.max)
        nc.vector.scalar_tensor_tensor(ag[:, sl], ag[:, sl], 0.0, tq[:, sl],
                                       ALU.max, ALU.mult)
        # denom = ag + ap - i2 + EPS ; iou = i2 / denom * fg
        nc.vector.tensor_tensor(ag[:, sl], ag[:, sl], preda_sb[:, sl], ALU.add)
        nc.vector.tensor_tensor(ag[:, sl], ag[:, sl], i2[:, sl], ALU.subtract)
        nc.vector.tensor_scalar(ag[:, sl], ag[:, sl], EPS, None, ALU.add)
        nc.vector.reciprocal(ag[:, sl], ag[:, sl])
        nc.vector.tensor_tensor(i2[:, sl], i2[:, sl], ag[:, sl], ALU.mult)
        nc.vector.tensor_tensor(iouu[:, sl], i2[:, sl], fg, ALU.mult)

        # scores
        for c in range(ca, cb):
            sc = scst.tile([128, BPC, NCLS], F32, tag="sc")
            for i in range(BPC):
                eng = nc.vector if i < 1 else nc.gpsimd
                eng.tensor_scalar(
                    sc[:, i, :], iota_sb, labf[:, c, i:i + 1],
                    iouu[:, c, i:i + 1], ALU.is_equal, ALU.mult)
            deng = nc.sync if (c % 2 == 0) else nc.scalar
            deng.dma_start(scores_o[:, c, :, :], sc[:])


# ---------------- host-side glue ----------------

_CACHE: dict = {}


def _prep_core_inputs(priors, gt_bboxes, gt_labels, pad_flag, pred_bboxes):
    """Build per-core input dicts (numpy only reshapes/replicates inputs)."""
    # priors -> [128, NCH, 10]: x1,y1,x2,y2,cx,cy,area,0,-cx,-cy (pad 1e6)
    ppad = np.full((PPAD, 4), 1.0e6, np.float32)
    ppad[:P] = priors
    px1, py1, px2, py2 = ppad[:, 0], ppad[:, 1], ppad[:, 2], ppad[:, 3]
    pcx = (px1 + px2) / np.float32(2.0)
    pcy = (py1 + py2) / np.float32(2.0)
    areap = (px2 - px1) * (py2 - py1)
    pr = np.stack([px1, py1, px2, py2, pcx, pcy, areap,
                   np.zeros_like(px1), -pcx, -pcy], axis=1)   # (PPAD, 10)
    pr = pr.reshape(NCH, 128, 10).transpose(1, 0, 2).copy()   # [r, c, 10]

    iota = np.broadcast_to(np.arange(NCLS, dtype=np.float32), (128, NCLS)).copy()
    ident = np.eye(128, dtype=np.float32)

    ins = []
    for core in range(NCORES):
        i0 = core * BPC
        gtb = gt_bboxes[i0:i0 + BPC].reshape(BG, 4).astype(np.float32)
        lab = gt_labels[i0:i0 + BPC].reshape(BG).astype(np.float32)
        padf = pad_flag[i0:i0 + BPC].reshape(BG, 1).astype(np.float32)
        gx1, gy1, gx2, gy2 = gtb[:, 0], gtb[:, 1], gtb[:, 2], gtb[:, 3]
        gcx = (gx1 + gx2) / np.float32(2.0)
        gcy = (gy1 + gy2) / np.float32(2.0)
        areag = (gx2 - gx1) * (gy2 - gy1)
        rows = np.stack([gx1, gy1, gx2, gy2, areag, gcx, gcy,
                         np.zeros_like(gx1)], axis=0)     # (8, BG)
        gtt = np.broadcast_to(rows[None], (128, 8, BG)).copy()

        # block-diag gather weights [BG, 6*BPC]: per img: x1,y1,x2,y2,label,1
        w = np.zeros((BG, 6 * BPC), np.float32)
        for i in range(BPC):
            sl = slice(i * G, (i + 1) * G)
            w[sl, 6 * i + 0:6 * i + 4] = gtb[sl]
            w[sl, 6 * i + 4] = lab[sl]
            w[sl, 6 * i + 5] = 1.0

        b0 = gt_bboxes[i0:i0 + BPC, 0, :].astype(np.float32).reshape(-1)  # (16,)
        box0 = np.broadcast_to(b0[None], (128, 4 * BPC)).copy()

        pb = pred_bboxes[i0:i0 + BPC].astype(np.float32)  # (BPC, P, 4)
        pbp = np.zeros((BPC, PPAD, 4), np.float32)
        pbp[:, :P] = pb
        predt = pbp.reshape(BPC, NCH, 128, 4).transpose(2, 1, 0, 3).copy()
        pa = (np.clip(pbp[..., 2] - pbp[..., 0], 0, None)
              * np.clip(pbp[..., 3] - pbp[..., 1], 0, None))  # (BPC, PPAD)
        preda = pa.reshape(BPC, NCH, 128).transpose(2, 1, 0).copy()

        parts = {
            "pr": pr, "gtt": gtt, "w": w, "pad": padf, "box0": box0,
            "predt": predt, "preda": preda, "iota80": iota, "ident": ident,
        }
        cstk = np.zeros((128, CST_W), np.float32)
        for name, _w in _CST_FIELDS:
            a, b = CST_OFF[name]
            cstk[:, a:b] = parts[name].reshape(128, b - a)
        ins.append({"cst": cstk})
    return ins


def kernel(priors, num_level_priors, gt_labels, gt_bboxes, pad_bbox_flag,
           pred_bboxes, _want_trace=False):
    priors = np.asarray(priors, np.float32)
    gt_bboxes = np.asarray(gt_bboxes, np.float32)
    gt_labels = np.asarray(gt_labels).reshape(B, G)
    pad_flag = np.asarray(pad_bbox_flag, np.float32).reshape(B, G)
    pred_bboxes = np.asarray(pred_bboxes, np.float32)

    if "nc" not in _CACHE:
        _CACHE["nc"] = _build_module()
    nc = _CACHE["nc"]

    in_maps = _prep_core_inputs(priors, gt_bboxes, gt_labels, pad_flag,
                                pred_bboxes)
    res = run_bass_kernel_spmd(nc, in_maps, core_ids=list(range(NCORES)),
                               trace=_want_trace)

    labels = np.empty((B, P), np.int32)
    boxes = np.empty((B, P, 4), np.float32)
    scores = np.empty((B, P, NCLS), np.float32)
    fg = np.empty((B, P), bool)
    for core in range(NCORES):
        r = res.results[core]
        i0 = core * BPC
        labv = r["labels_o"].transpose(2, 1, 0).reshape(BPC, PPAD)
        labels[i0:i0 + BPC] = labv[:, :P]
        bx = r["boxes_o"].transpose(2, 1, 0, 3).reshape(BPC, PPAD, 4)
        boxes[i0:i0 + BPC] = bx[:, :P]
        sc = r["scores_o"].transpose(2, 1, 0, 3).reshape(BPC, PPAD, NCLS)
        scores[i0:i0 + BPC] = sc[:, :P]
        f = r["fg_o"].transpose(2, 1, 0).reshape(BPC, PPAD)
        fg[i0:i0 + BPC] = f[:, :P] > 0
    out = (labels, boxes, scores, fg)
    if _want_trace:
        return out, res
    return out
